# revision 10
# baseline (speedup 1.0000x reference)
"""BAP classifier (attention-pooling + linear head) on 8 TRN2 NeuronCores.

Pipeline (reference math):
    A    = sigmoid(einsum('bchw,mc->bmhw', x, Wa) + ba)     # attention maps
    bap  = einsum('bchw,bmhw->bmc', x, A) / (H*W)           # attn-weighted pool
    out  = bap.reshape(B, M*C) @ Wc.T + bc                  # linear head

Sharding:
  Phase 1 — data-parallel over batch (8 batches/core): each core computes
    raw feats rows [8, M*C] (un-normalized bap, transposed per batch on chip).
  Phase 2 — Wc column-parallel (8192 columns of the M*C dim per core): each
    core computes a partial [B, NCLS] logit; host sums partials, applies the
    1/(H*W) scale and bias.

Compute dtype is bf16 on the TensorEngine with fp32 PSUM accumulation.

Schedule notes (from NTFF traces):
  - each dma_start costs ~0.7us of issue time on its queue engine and a
    single queue sustains well under HBM rate, so transfers are spread
    over the sync/scalar (HWDGE) and gpsimd (SWDGE) queues;
  - the PE clock (HAM) throttles to 1.2 GHz after ~3.4us of idle, so the
    PE is kept warm with junk matmuls during the initial DMA ramp and the
    program order interleaves independent work into every dependency gap;
  - PSUM->SBUF drains split across Scalar and Vector so neither gates the
    bap accumulation banks.
"""
import sys

if "/opt/trn_rl_repo" not in sys.path:
    sys.path.insert(0, "/opt/trn_rl_repo")

import numpy as np

import concourse.bacc as bacc
import concourse.mybir as mybir
from concourse.tile import TileContext
from concourse.bass_utils import run_bass_kernel_spmd
from concourse.masks import make_identity

B, C, H, W = 64, 2048, 14, 14
HW = H * W                     # 196
M, NCLS = 32, 396
NCORES = 8
BPC = B // NCORES              # 8 batches per core
CT = C // 128                  # 16 c-chunks
KTOT = M * C                   # 65536
KPC = KTOT // NCORES           # 8192 Wc columns per core
KT = KPC // 128                # 64 k-tiles per core in phase 2

F32 = mybir.dt.float32
BF16 = mybir.dt.bfloat16

# Run options (test harness may flip these; defaults are what grading uses).
TRACE = False
TRACE_INFO = {}
TRACE_RES = {}

_cache = {}


def _nc():
    return bacc.Bacc(
        "TRN2", target_bir_lowering=False, debug=False, num_devices=NCORES
    )


def _build_phase1():
    """Per-core: x shard -> raw feats [BPC, M*C] (bf16).

    Inputs (host-permuted so every DMA descriptor is a contiguous >=4KB run):
      xp  [128, 4, CT, 2, HW]  batch pairs, c = p*CT + t
      xta [128, BPC, C]        x^T rows hw=0:128
      xtb [ 68, BPC, C]        x^T rows hw=128:196
      wat [128, CT, M]         Wa^T in the same permuted-c layout
      ba  [M, 1]

    All loads ride the sync queue in exact consumption order (a single
    HWDGE queue sustains the full ~360 GB/s); feats stores ride gpsimd and
    are emitted last so no load ever recycles a store's DMA semaphore
    (the pool has only ~8 and a load waiting on a compute-gated store
    stalls the whole ring).
    """
    nc = _nc()
    xp = nc.dram_tensor("xp", [128, 4, CT, 2, HW], BF16, kind="ExternalInput")
    xta = nc.dram_tensor("xta", [128, BPC, C], BF16, kind="ExternalInput")
    xtb = nc.dram_tensor("xtb", [68, BPC, C], BF16, kind="ExternalInput")
    wat = nc.dram_tensor("wat", [128, CT, M], BF16, kind="ExternalInput")
    ba = nc.dram_tensor("ba", [M, 1], F32, kind="ExternalInput")
    feats = nc.dram_tensor("feats", [BPC, M * C], BF16, kind="ExternalOutput")

    with TileContext(nc) as tc:
        with (
            tc.tile_pool(name="const", bufs=1) as const,
            tc.tile_pool(name="xpool", bufs=3) as xpool,
            tc.tile_pool(name="xtapool", bufs=3) as xtapool,
            tc.tile_pool(name="xtbpool", bufs=3) as xtbpool,
            tc.tile_pool(name="apool", bufs=3) as apool,
            tc.tile_pool(name="atpool", bufs=4) as atpool,
            tc.tile_pool(name="fpool", bufs=2) as fpool,
            tc.tile_pool(name="ps_att", bufs=2, space="PSUM") as ps_att,
            tc.tile_pool(name="ps_tr", bufs=1, space="PSUM") as ps_tr,
            tc.tile_pool(name="ps_bap", bufs=2, space="PSUM") as ps_bap,
        ):
            # PE warm-up source (memset on gpsimd before identity/stores)
            warm_sb = const.tile([128, 512], BF16)
            nc.gpsimd.memset(warm_sb, 0.0)
            ident = const.tile([M, M], BF16)
            make_identity(nc, ident)

            # loads, sync queue, consumption order
            wat_sb = const.tile([128, CT, M], BF16)
            nc.sync.dma_start(out=wat_sb, in_=wat.ap())
            ba_sb = const.tile([M, 1], F32)
            nc.sync.dma_start(out=ba_sb, in_=ba.ap())
            xps, xtas, xtbs = [], [], []
            for p in range(4):
                x_p = xpool.tile(
                    [128, CT, 2, HW], BF16, tag="xp", name=f"xp{p}"
                )
                nc.sync.dma_start(out=x_p, in_=xp.ap()[:, p])
                xps.append(x_p)
                xta_p = xtapool.tile(
                    [128, 2, C], BF16, tag="xta", name=f"xta{p}"
                )
                nc.sync.dma_start(
                    out=xta_p, in_=xta.ap()[:, 2 * p : 2 * p + 2]
                )
                xtb_p = xtbpool.tile([68, 2, C], BF16, tag="xtb", name=f"xtb{p}")
                nc.sync.dma_start(
                    out=xtb_p, in_=xtb.ap()[:, 2 * p : 2 * p + 2]
                )
                xtas.append(xta_p)
                xtbs.append(xtb_p)

            # keep the PE busy (and the HAM clock-gate open) during the
            # initial DMA ramp: one long junk accumulation chain.
            w_ps = ps_att.tile([128, 512], F32, tag="att", name="warm")
            for i in range(16):
                nc.tensor.matmul(
                    w_ps, lhsT=warm_sb[:, 0:128], rhs=warm_sb,
                    start=(i == 0), stop=(i == 15),
                )

            a_of = {}

            def emit_e1_pair(i):
                att_ps = ps_att.tile([M, 2, HW], F32, tag="att", name=f"attp{i}")
                for ct in range(CT):
                    nc.tensor.matmul(
                        att_ps,
                        lhsT=wat_sb[:, ct, :],
                        rhs=xps[i][:, ct, :, :],
                        start=(ct == 0),
                        stop=(ct == CT - 1),
                    )
                a_sb = apool.tile(
                    [M, 2, HW], BF16, tag="a_sb", name=f"a_sbp{i}"
                )
                nc.scalar.activation(
                    out=a_sb, in_=att_ps,
                    func=mybir.ActivationFunctionType.Sigmoid, bias=ba_sb,
                )
                a_of[2 * i] = (a_sb, 0)
                a_of[2 * i + 1] = (a_sb, 1)

            ats = {}

            def emit_tr(b):
                a_sb, b2 = a_of[b]
                ata_ps = ps_tr.tile([128, M], BF16, tag="ata")
                nc.tensor.transpose(ata_ps, a_sb[:, b2, 0:128], ident)
                ata = atpool.tile([128, M], BF16, tag="ata_sb")
                nc.scalar.copy(out=ata, in_=ata_ps)
                atb_ps = ps_tr.tile([68, M], BF16, tag="atb")
                nc.tensor.transpose(atb_ps, a_sb[:, b2, 128:HW], ident)
                atb = atpool.tile([68, M], BF16, tag="atb_sb")
                nc.vector.tensor_copy(out=atb, in_=atb_ps)
                ats[b] = (ata, atb)

            fqs = {}

            def emit_e2(b):
                ata, atb = ats[b]
                xta_b = xtas[b // 2][:, b % 2]
                xtb_b = xtbs[b // 2][:, b % 2]
                if b % 2 == 0:
                    fqs[b // 2] = fpool.tile(
                        [2 * M, C], BF16, tag="featsq", name=f"fq{b // 2}"
                    )
                featsq = fqs[b // 2]
                r0 = M * (b % 2)
                bap_ps = [
                    ps_bap.tile([M, 1024], F32, tag="bap", name=f"bap{b}_{h}")
                    for h in range(2)
                ]
                for h in range(2):
                    for nt in range(2):
                        c0 = 1024 * h + 512 * nt
                        nc.tensor.matmul(
                            bap_ps[h][:, 512 * nt : 512 * (nt + 1)],
                            lhsT=ata, rhs=xta_b[:, c0 : c0 + 512],
                            start=True, stop=False,
                        )
                for h in range(2):
                    for nt in range(2):
                        c0 = 1024 * h + 512 * nt
                        nc.tensor.matmul(
                            bap_ps[h][:, 512 * nt : 512 * (nt + 1)],
                            lhsT=atb, rhs=xtb_b[:, c0 : c0 + 512],
                            start=False, stop=True,
                        )
                # parallel drains: vector gets half 0, scalar gets half 1
                nc.vector.tensor_copy(
                    out=featsq[r0 : r0 + M, 0:1024], in_=bap_ps[0]
                )
                nc.scalar.copy(
                    out=featsq[r0 : r0 + M, 1024:2048], in_=bap_ps[1]
                )

            # PE program order: einsum1 of the next pair is interleaved so
            # the PE never waits on Scalar/Vector round trips.
            emit_e1_pair(0)
            emit_tr(0)
            emit_e2(0)
            emit_tr(1)
            emit_e2(1)
            emit_e1_pair(1)
            emit_tr(2)
            emit_e2(2)
            emit_tr(3)
            emit_e2(3)
            emit_e1_pair(2)
            emit_tr(4)
            emit_e2(4)
            emit_tr(5)
            emit_e2(5)
            emit_e1_pair(3)
            emit_tr(6)
            emit_e2(6)
            emit_tr(7)
            emit_e2(7)

            # pair stores, gpsimd queue, emitted after every load
            for p in range(4):
                nc.gpsimd.dma_start(
                    out=feats.ap()[2 * p : 2 * p + 2].rearrange(
                        "b (m c) -> (b m) c", m=M
                    ),
                    in_=fqs[p],
                )
    nc.compile()
    return nc


def _build_phase2():
    """Per-core: featsT slice (partition-major, bf16) x WcT slice (bf16)
    -> partial [B, NCLS] (fp32)."""
    nc = _nc()
    ft = nc.dram_tensor("ft", [128, KT, B], BF16, kind="ExternalInput")
    wct = nc.dram_tensor("wct", [128, KT, NCLS], BF16, kind="ExternalInput")
    part = nc.dram_tensor("part", [B, NCLS], F32, kind="ExternalOutput")

    # graduated chunk sizes so the first matmul starts early; everything
    # on the sync queue in exact consumption order (ft pieces interleaved
    # right before the chunks that need them; no chunk crosses an ft
    # boundary).  Output store on scalar so it can't block the load ring.
    CHUNKS = [2, 2, 4, 4, 4, 8, 8, 8, 8, 8, 6, 2]
    FSTEP = 16

    with TileContext(nc) as tc:
        with (
            tc.tile_pool(name="const", bufs=1) as const,
            tc.tile_pool(name="fpool", bufs=4) as fpool,
            tc.tile_pool(name="wpool", bufs=len(CHUNKS)) as wpool,
            tc.tile_pool(name="opool", bufs=1) as opool,
            tc.tile_pool(name="ps_out", bufs=1, space="PSUM") as ps_out,
            tc.tile_pool(name="ps_warm", bufs=1, space="PSUM") as ps_warm,
        ):
            warm_sb = const.tile([128, 512], BF16)
            nc.gpsimd.memset(warm_sb, 0.0)

            ft_sb = []

            def load_ft(i):
                t = fpool.tile([128, FSTEP, B], BF16, tag="ft", name=f"ft{i}")
                nc.sync.dma_start(
                    out=t, in_=ft.ap()[:, i * FSTEP : (i + 1) * FSTEP]
                )
                ft_sb.append(t)

            load_ft(0)
            w_sbs = []
            k0 = 0
            for kc, ch in enumerate(CHUNKS):
                if k0 + ch > FSTEP * len(ft_sb):
                    load_ft(len(ft_sb))
                w_sb = wpool.tile(
                    [128, ch, NCLS], BF16, tag=f"w{kc}", bufs=1, name=f"w{kc}"
                )
                nc.sync.dma_start(out=w_sb, in_=wct.ap()[:, k0 : k0 + ch])
                w_sbs.append((k0, w_sb))
                k0 += ch

            w_ps = ps_warm.tile([128, 512], F32, tag="warm", name="wm")
            for i in range(14):
                nc.tensor.matmul(
                    w_ps, lhsT=warm_sb[:, 0:128], rhs=warm_sb,
                    start=(i == 0), stop=(i == 13),
                )

            out_ps = ps_out.tile([B, NCLS], F32)
            for k0, w_sb in w_sbs:
                for kl in range(w_sb.shape[1]):
                    kt = k0 + kl
                    nc.tensor.matmul(
                        out_ps,
                        lhsT=ft_sb[kt // FSTEP][:, kt % FSTEP, :],
                        rhs=w_sb[:, kl, :],
                        start=(kt == 0),
                        stop=(kt == KT - 1),
                    )
            out_sb = opool.tile([B, NCLS], F32)
            nc.vector.tensor_copy(out=out_sb, in_=out_ps)
            nc.scalar.dma_start(out=part.ap(), in_=out_sb)
    nc.compile()
    return nc


def _install_ntff_hook():
    import types

    import trn_agent_boot.trn_boot as tb
    import concourse.bass_utils as bu

    hook = tb._ntff_profile_via_ctypes("/opt/axon/libaxon_pjrt.so")
    mod = types.ModuleType("antenv.axon_hooks")
    mod.get_axon_ntff_profile_hook = lambda: hook
    sys.modules["antenv.axon_hooks"] = mod
    bu.upload_artifacts = lambda tmpdir: "(skipped)"


def _run(nc, in_maps, label):
    core_ids = list(range(NCORES))
    if TRACE:
        _install_ntff_hook()
        res = run_bass_kernel_spmd(nc, in_maps, core_ids, trace=True)
        TRACE_INFO[label] = res.exec_time_ns
        TRACE_RES[label] = res
    else:
        res = run_bass_kernel_spmd(nc, in_maps, core_ids)
    return res.results


def kernel(x, Wa, ba, Wc, bc):
    import ml_dtypes

    bf16 = np.dtype(ml_dtypes.bfloat16)
    x3 = np.ascontiguousarray(x, dtype=np.float32).reshape(B, C, HW)
    xb = x3.astype(bf16)
    x4 = xb.reshape(B, 128, CT, HW)
    # xp[p, i, t, b2, hw] with c = p*CT + t
    xps = [
        np.ascontiguousarray(
            x4[i * BPC : (i + 1) * BPC]
            .reshape(4, 2, 128, CT, HW)
            .transpose(2, 0, 3, 1, 4)
        )
        for i in range(NCORES)
    ]
    xt = xb.transpose(2, 0, 1)  # [HW, B, C]
    xta = np.ascontiguousarray(xt[0:128])
    xtb = np.ascontiguousarray(xt[128:196])
    # wat[p, t, m] = Wa[m, p*CT + t] — matches the kernel's permuted c layout
    wat = np.ascontiguousarray(Wa.T, dtype=np.float32).astype(bf16).reshape(
        128, CT, M
    )
    ba2 = np.ascontiguousarray(ba, dtype=np.float32).reshape(M, 1)
    wct = np.ascontiguousarray(Wc.T, dtype=np.float32).astype(bf16)  # [KTOT, NCLS]

    if "p1" not in _cache:
        _cache["p1"] = _build_phase1()
    if "p2" not in _cache:
        _cache["p2"] = _build_phase2()

    in1 = [
        {
            "xp": xps[i],
            "xta": xta[:, i * BPC : (i + 1) * BPC],
            "xtb": xtb[:, i * BPC : (i + 1) * BPC],
            "wat": wat,
            "ba": ba2,
        }
        for i in range(NCORES)
    ]
    res1 = _run(_cache["p1"], in1, "phase1")
    feats = np.concatenate([r["feats"] for r in res1], axis=0)  # [B, KTOT] bf16

    # ft[p, t, b] = feats[b, kslice + t*128 + p] (partition-major, bf16)
    featsT = np.ascontiguousarray(feats.T)  # [KTOT, B]
    in2 = [
        {
            "ft": np.ascontiguousarray(
                featsT[i * KPC : (i + 1) * KPC].reshape(KT, 128, B).transpose(
                    1, 0, 2
                )
            ),
            "wct": np.ascontiguousarray(
                wct[i * KPC : (i + 1) * KPC].reshape(KT, 128, NCLS).transpose(
                    1, 0, 2
                )
            ),
        }
        for i in range(NCORES)
    ]
    res2 = _run(_cache["p2"], in2, "phase2")
    parts = np.stack([r["part"] for r in res2], axis=0)  # [NCORES, B, NCLS]

    logits = parts.sum(axis=0) / float(HW) + np.asarray(bc, dtype=np.float32)
    return logits.astype(np.float32)


# revision 11
# speedup vs baseline: 1.0071x; 1.0071x over previous
"""BAP classifier (attention-pooling + linear head) on 8 TRN2 NeuronCores.

Pipeline (reference math):
    A    = sigmoid(einsum('bchw,mc->bmhw', x, Wa) + ba)     # attention maps
    bap  = einsum('bchw,bmhw->bmc', x, A) / (H*W)           # attn-weighted pool
    out  = bap.reshape(B, M*C) @ Wc.T + bc                  # linear head

Sharding:
  Phase 1 — data-parallel over batch (8 batches/core): each core computes
    raw feats rows [8, M*C] (un-normalized bap, transposed per batch on chip).
  Phase 2 — Wc column-parallel (8192 columns of the M*C dim per core): each
    core computes a partial [B, NCLS] logit; host sums partials, applies the
    1/(H*W) scale and bias.

Compute dtype is bf16 on the TensorEngine with fp32 PSUM accumulation.

Schedule notes (from NTFF traces):
  - each dma_start costs ~0.7us of issue time on its queue engine and a
    single queue sustains well under HBM rate, so transfers are spread
    over the sync/scalar (HWDGE) and gpsimd (SWDGE) queues;
  - the PE clock (HAM) throttles to 1.2 GHz after ~3.4us of idle, so the
    PE is kept warm with junk matmuls during the initial DMA ramp and the
    program order interleaves independent work into every dependency gap;
  - PSUM->SBUF drains split across Scalar and Vector so neither gates the
    bap accumulation banks.
"""
import sys

if "/opt/trn_rl_repo" not in sys.path:
    sys.path.insert(0, "/opt/trn_rl_repo")

import numpy as np

import concourse.bacc as bacc
import concourse.mybir as mybir
from concourse.tile import TileContext
from concourse.bass_utils import run_bass_kernel_spmd
from concourse.masks import make_identity

B, C, H, W = 64, 2048, 14, 14
HW = H * W                     # 196
M, NCLS = 32, 396
NCORES = 8
BPC = B // NCORES              # 8 batches per core
CT = C // 128                  # 16 c-chunks
KTOT = M * C                   # 65536
KPC = KTOT // NCORES           # 8192 Wc columns per core
KT = KPC // 128                # 64 k-tiles per core in phase 2

F32 = mybir.dt.float32
BF16 = mybir.dt.bfloat16

# Run options (test harness may flip these; defaults are what grading uses).
TRACE = False
TRACE_INFO = {}
TRACE_RES = {}

_cache = {}


def _nc():
    return bacc.Bacc(
        "TRN2", target_bir_lowering=False, debug=False, num_devices=NCORES
    )


def _build_phase1():
    """Per-core: x shard -> raw feats [BPC, M*C] (bf16).

    Inputs (host-permuted so every DMA descriptor is a contiguous >=4KB run):
      xp  [128, 4, CT, 2, HW]  batch pairs, c = p*CT + t
      xta [128, BPC, C]        x^T rows hw=0:128
      xtb [ 68, BPC, C]        x^T rows hw=128:196
      wat [128, CT, M]         Wa^T in the same permuted-c layout
      ba  [M, 1]

    All loads ride the sync queue in exact consumption order (a single
    HWDGE queue sustains the full ~360 GB/s); feats stores ride gpsimd and
    are emitted last so no load ever recycles a store's DMA semaphore
    (the pool has only ~8 and a load waiting on a compute-gated store
    stalls the whole ring).
    """
    nc = _nc()
    xp = nc.dram_tensor("xp", [128, 4, CT, 2, HW], BF16, kind="ExternalInput")
    xta = nc.dram_tensor("xta", [128, BPC, C], BF16, kind="ExternalInput")
    xtb = nc.dram_tensor("xtb", [68, BPC, C], BF16, kind="ExternalInput")
    wat = nc.dram_tensor("wat", [128, CT, M], BF16, kind="ExternalInput")
    ba = nc.dram_tensor("ba", [M, 1], F32, kind="ExternalInput")
    feats = nc.dram_tensor("feats", [BPC, M * C], BF16, kind="ExternalOutput")

    with TileContext(nc) as tc:
        with (
            tc.tile_pool(name="const", bufs=1) as const,
            tc.tile_pool(name="xpool", bufs=3) as xpool,
            tc.tile_pool(name="xtapool", bufs=3) as xtapool,
            tc.tile_pool(name="xtbpool", bufs=3) as xtbpool,
            tc.tile_pool(name="apool", bufs=3) as apool,
            tc.tile_pool(name="atpool", bufs=4) as atpool,
            tc.tile_pool(name="fpool", bufs=2) as fpool,
            tc.tile_pool(name="ps_att", bufs=2, space="PSUM") as ps_att,
            tc.tile_pool(name="ps_tr", bufs=1, space="PSUM") as ps_tr,
            tc.tile_pool(name="ps_bap", bufs=2, space="PSUM") as ps_bap,
        ):
            # PE warm-up source (memset on gpsimd before identity/stores)
            warm_sb = const.tile([128, 512], BF16)
            nc.gpsimd.memset(warm_sb, 0.0)
            ident = const.tile([M, M], BF16)
            make_identity(nc, ident)

            # loads, sync queue, consumption order
            wat_sb = const.tile([128, CT, M], BF16)
            nc.sync.dma_start(out=wat_sb, in_=wat.ap())
            ba_sb = const.tile([M, 1], F32)
            nc.sync.dma_start(out=ba_sb, in_=ba.ap())
            xps, xtas, xtbs = [], [], []
            for p in range(4):
                x_p = xpool.tile(
                    [128, CT, 2, HW], BF16, tag="xp", name=f"xp{p}"
                )
                nc.sync.dma_start(out=x_p, in_=xp.ap()[:, p])
                xps.append(x_p)
                xta_p = xtapool.tile(
                    [128, 2, C], BF16, tag="xta", name=f"xta{p}"
                )
                nc.sync.dma_start(
                    out=xta_p, in_=xta.ap()[:, 2 * p : 2 * p + 2]
                )
                # xtb covers only 68 partitions (~9/16 SDMA engines), so it
                # rides the scalar queue concurrently with the sync stream
                xtb_p = xtbpool.tile([68, 2, C], BF16, tag="xtb", name=f"xtb{p}")
                nc.scalar.dma_start(
                    out=xtb_p, in_=xtb.ap()[:, 2 * p : 2 * p + 2]
                )
                xtas.append(xta_p)
                xtbs.append(xtb_p)

            # keep the PE busy (and the HAM clock-gate open) during the
            # initial DMA ramp: one long junk accumulation chain.
            w_ps = ps_att.tile([128, 512], F32, tag="att", name="warm")
            for i in range(16):
                nc.tensor.matmul(
                    w_ps, lhsT=warm_sb[:, 0:128], rhs=warm_sb,
                    start=(i == 0), stop=(i == 15),
                )

            a_of = {}

            def emit_e1_pair(i):
                att_ps = ps_att.tile([M, 2, HW], F32, tag="att", name=f"attp{i}")
                for ct in range(CT):
                    nc.tensor.matmul(
                        att_ps,
                        lhsT=wat_sb[:, ct, :],
                        rhs=xps[i][:, ct, :, :],
                        start=(ct == 0),
                        stop=(ct == CT - 1),
                    )
                a_sb = apool.tile(
                    [M, 2, HW], BF16, tag="a_sb", name=f"a_sbp{i}"
                )
                nc.scalar.activation(
                    out=a_sb, in_=att_ps,
                    func=mybir.ActivationFunctionType.Sigmoid, bias=ba_sb,
                )
                a_of[2 * i] = (a_sb, 0)
                a_of[2 * i + 1] = (a_sb, 1)

            ats = {}

            def emit_tr(b):
                a_sb, b2 = a_of[b]
                ata_ps = ps_tr.tile([128, M], BF16, tag="ata")
                nc.tensor.transpose(ata_ps, a_sb[:, b2, 0:128], ident)
                ata = atpool.tile([128, M], BF16, tag="ata_sb")
                nc.scalar.copy(out=ata, in_=ata_ps)
                atb_ps = ps_tr.tile([68, M], BF16, tag="atb")
                nc.tensor.transpose(atb_ps, a_sb[:, b2, 128:HW], ident)
                atb = atpool.tile([68, M], BF16, tag="atb_sb")
                nc.vector.tensor_copy(out=atb, in_=atb_ps)
                ats[b] = (ata, atb)

            fqs = {}

            def emit_e2(b):
                ata, atb = ats[b]
                xta_b = xtas[b // 2][:, b % 2]
                xtb_b = xtbs[b // 2][:, b % 2]
                if b % 4 == 0:
                    fqs[b // 4] = fpool.tile(
                        [4 * M, C], BF16, tag="featsq", name=f"fq{b // 4}"
                    )
                featsq = fqs[b // 4]
                r0 = M * (b % 4)
                bap_ps = [
                    ps_bap.tile([M, 1024], F32, tag="bap", name=f"bap{b}_{h}")
                    for h in range(2)
                ]
                for h in range(2):
                    for nt in range(2):
                        c0 = 1024 * h + 512 * nt
                        nc.tensor.matmul(
                            bap_ps[h][:, 512 * nt : 512 * (nt + 1)],
                            lhsT=ata, rhs=xta_b[:, c0 : c0 + 512],
                            start=True, stop=False,
                        )
                for h in range(2):
                    for nt in range(2):
                        c0 = 1024 * h + 512 * nt
                        nc.tensor.matmul(
                            bap_ps[h][:, 512 * nt : 512 * (nt + 1)],
                            lhsT=atb, rhs=xtb_b[:, c0 : c0 + 512],
                            start=False, stop=True,
                        )
                # parallel drains: vector gets half 0, scalar gets half 1
                nc.vector.tensor_copy(
                    out=featsq[r0 : r0 + M, 0:1024], in_=bap_ps[0]
                )
                nc.scalar.copy(
                    out=featsq[r0 : r0 + M, 1024:2048], in_=bap_ps[1]
                )

            # PE program order: einsum1 of the next pair is interleaved so
            # the PE never waits on Scalar/Vector round trips.
            emit_e1_pair(0)
            emit_tr(0)
            emit_e2(0)
            emit_tr(1)
            emit_e2(1)
            emit_e1_pair(1)
            emit_tr(2)
            emit_e2(2)
            emit_tr(3)
            emit_e2(3)
            emit_e1_pair(2)
            emit_tr(4)
            emit_e2(4)
            emit_tr(5)
            emit_e2(5)
            emit_e1_pair(3)
            emit_tr(6)
            emit_e2(6)
            emit_tr(7)
            emit_e2(7)

            # quad stores (128 partitions, 4KB descriptors), gpsimd queue,
            # emitted after every load
            for q in range(2):
                nc.gpsimd.dma_start(
                    out=feats.ap()[4 * q : 4 * q + 4].rearrange(
                        "b (m c) -> (b m) c", m=M
                    ),
                    in_=fqs[q],
                )
    nc.compile()
    return nc


def _build_phase2():
    """Per-core: featsT slice (partition-major, bf16) x WcT slice (bf16)
    -> partial [B, NCLS] (fp32)."""
    nc = _nc()
    ft = nc.dram_tensor("ft", [128, KT, B], BF16, kind="ExternalInput")
    wct = nc.dram_tensor("wct", [128, KT, NCLS], BF16, kind="ExternalInput")
    part = nc.dram_tensor("part", [B, NCLS], F32, kind="ExternalOutput")

    # 16-k-tile chunks give 12.7KB per-partition descriptors (near line
    # rate per SDMA engine); the last chunks shrink so the final
    # chunk-completion latency + its matmuls are a short tail.  All loads
    # on sync in consumption order; ft halves interleaved.
    CHUNKS = [16, 16, 16, 8, 4, 2, 2]
    FSTEP = 32

    with TileContext(nc) as tc:
        with (
            tc.tile_pool(name="const", bufs=1) as const,
            tc.tile_pool(name="fpool", bufs=2) as fpool,
            tc.tile_pool(name="wpool", bufs=len(CHUNKS)) as wpool,
            tc.tile_pool(name="opool", bufs=1) as opool,
            tc.tile_pool(name="ps_out", bufs=1, space="PSUM") as ps_out,
            tc.tile_pool(name="ps_warm", bufs=1, space="PSUM") as ps_warm,
        ):
            warm_sb = const.tile([128, 512], BF16)
            nc.gpsimd.memset(warm_sb, 0.0)

            ft_sb = []

            def load_ft(i):
                t = fpool.tile([128, FSTEP, B], BF16, tag="ft", name=f"ft{i}")
                nc.sync.dma_start(
                    out=t, in_=ft.ap()[:, i * FSTEP : (i + 1) * FSTEP]
                )
                ft_sb.append(t)

            load_ft(0)
            w_sbs = []
            k0 = 0
            for kc, ch in enumerate(CHUNKS):
                if k0 + ch > FSTEP * len(ft_sb):
                    load_ft(len(ft_sb))
                w_sb = wpool.tile(
                    [128, ch, NCLS], BF16, tag=f"w{kc}", bufs=1, name=f"w{kc}"
                )
                nc.sync.dma_start(out=w_sb, in_=wct.ap()[:, k0 : k0 + ch])
                w_sbs.append((k0, w_sb))
                k0 += ch

            # junk accumulation chain keeps the PE warm until the first
            # (16-k-tile) chunk lands
            w_ps = ps_warm.tile([128, 512], F32, tag="warm", name="wm")
            for i in range(26):
                nc.tensor.matmul(
                    w_ps, lhsT=warm_sb[:, 0:128], rhs=warm_sb,
                    start=(i == 0), stop=(i == 25),
                )

            out_ps = ps_out.tile([B, NCLS], F32)
            for k0, w_sb in w_sbs:
                for kl in range(w_sb.shape[1]):
                    kt = k0 + kl
                    nc.tensor.matmul(
                        out_ps,
                        lhsT=ft_sb[kt // FSTEP][:, kt % FSTEP, :],
                        rhs=w_sb[:, kl, :],
                        start=(kt == 0),
                        stop=(kt == KT - 1),
                    )
            out_sb = opool.tile([B, NCLS], F32)
            nc.vector.tensor_copy(out=out_sb, in_=out_ps)
            nc.scalar.dma_start(out=part.ap(), in_=out_sb)
    nc.compile()
    return nc


def _install_ntff_hook():
    import types

    import trn_agent_boot.trn_boot as tb
    import concourse.bass_utils as bu

    hook = tb._ntff_profile_via_ctypes("/opt/axon/libaxon_pjrt.so")
    mod = types.ModuleType("antenv.axon_hooks")
    mod.get_axon_ntff_profile_hook = lambda: hook
    sys.modules["antenv.axon_hooks"] = mod
    bu.upload_artifacts = lambda tmpdir: "(skipped)"


def _run(nc, in_maps, label):
    core_ids = list(range(NCORES))
    if TRACE:
        _install_ntff_hook()
        res = run_bass_kernel_spmd(nc, in_maps, core_ids, trace=True)
        TRACE_INFO[label] = res.exec_time_ns
        TRACE_RES[label] = res
    else:
        res = run_bass_kernel_spmd(nc, in_maps, core_ids)
    return res.results


def kernel(x, Wa, ba, Wc, bc):
    import ml_dtypes

    bf16 = np.dtype(ml_dtypes.bfloat16)
    x3 = np.ascontiguousarray(x, dtype=np.float32).reshape(B, C, HW)
    xb = x3.astype(bf16)
    x4 = xb.reshape(B, 128, CT, HW)
    # xp[p, i, t, b2, hw] with c = p*CT + t
    xps = [
        np.ascontiguousarray(
            x4[i * BPC : (i + 1) * BPC]
            .reshape(4, 2, 128, CT, HW)
            .transpose(2, 0, 3, 1, 4)
        )
        for i in range(NCORES)
    ]
    xt = xb.transpose(2, 0, 1)  # [HW, B, C]
    xta = np.ascontiguousarray(xt[0:128])
    xtb = np.ascontiguousarray(xt[128:196])
    # wat[p, t, m] = Wa[m, p*CT + t] — matches the kernel's permuted c layout
    wat = np.ascontiguousarray(Wa.T, dtype=np.float32).astype(bf16).reshape(
        128, CT, M
    )
    ba2 = np.ascontiguousarray(ba, dtype=np.float32).reshape(M, 1)
    wct = np.ascontiguousarray(Wc.T, dtype=np.float32).astype(bf16)  # [KTOT, NCLS]

    if "p1" not in _cache:
        _cache["p1"] = _build_phase1()
    if "p2" not in _cache:
        _cache["p2"] = _build_phase2()

    in1 = [
        {
            "xp": xps[i],
            "xta": xta[:, i * BPC : (i + 1) * BPC],
            "xtb": xtb[:, i * BPC : (i + 1) * BPC],
            "wat": wat,
            "ba": ba2,
        }
        for i in range(NCORES)
    ]
    res1 = _run(_cache["p1"], in1, "phase1")
    feats = np.concatenate([r["feats"] for r in res1], axis=0)  # [B, KTOT] bf16

    # ft[p, t, b] = feats[b, kslice + t*128 + p] (partition-major, bf16)
    featsT = np.ascontiguousarray(feats.T)  # [KTOT, B]
    in2 = [
        {
            "ft": np.ascontiguousarray(
                featsT[i * KPC : (i + 1) * KPC].reshape(KT, 128, B).transpose(
                    1, 0, 2
                )
            ),
            "wct": np.ascontiguousarray(
                wct[i * KPC : (i + 1) * KPC].reshape(KT, 128, NCLS).transpose(
                    1, 0, 2
                )
            ),
        }
        for i in range(NCORES)
    ]
    res2 = _run(_cache["p2"], in2, "phase2")
    parts = np.stack([r["part"] for r in res2], axis=0)  # [NCORES, B, NCLS]

    logits = parts.sum(axis=0) / float(HW) + np.asarray(bc, dtype=np.float32)
    return logits.astype(np.float32)


# revision 12
# speedup vs baseline: 1.0114x; 1.0043x over previous
"""BAP classifier (attention-pooling + linear head) on 8 TRN2 NeuronCores.

Pipeline (reference math):
    A    = sigmoid(einsum('bchw,mc->bmhw', x, Wa) + ba)     # attention maps
    bap  = einsum('bchw,bmhw->bmc', x, A) / (H*W)           # attn-weighted pool
    out  = bap.reshape(B, M*C) @ Wc.T + bc                  # linear head

Sharding:
  Phase 1 — data-parallel over batch (8 batches/core): each core computes
    raw feats rows [8, M*C] (un-normalized bap, transposed per batch on chip).
  Phase 2 — Wc column-parallel (8192 columns of the M*C dim per core): each
    core computes a partial [B, NCLS] logit; host sums partials, applies the
    1/(H*W) scale and bias.

Compute dtype is bf16 on the TensorEngine with fp32 PSUM accumulation.

Schedule notes (from NTFF traces):
  - each dma_start costs ~0.7us of issue time on its queue engine and a
    single queue sustains well under HBM rate, so transfers are spread
    over the sync/scalar (HWDGE) and gpsimd (SWDGE) queues;
  - the PE clock (HAM) throttles to 1.2 GHz after ~3.4us of idle, so the
    PE is kept warm with junk matmuls during the initial DMA ramp and the
    program order interleaves independent work into every dependency gap;
  - PSUM->SBUF drains split across Scalar and Vector so neither gates the
    bap accumulation banks.
"""
import sys

if "/opt/trn_rl_repo" not in sys.path:
    sys.path.insert(0, "/opt/trn_rl_repo")

import numpy as np

import concourse.bacc as bacc
import concourse.mybir as mybir
from concourse.tile import TileContext
from concourse.bass_utils import run_bass_kernel_spmd
from concourse.masks import make_identity

B, C, H, W = 64, 2048, 14, 14
HW = H * W                     # 196
M, NCLS = 32, 396
NCORES = 8
BPC = B // NCORES              # 8 batches per core
CT = C // 128                  # 16 c-chunks
KTOT = M * C                   # 65536
KPC = KTOT // NCORES           # 8192 Wc columns per core
KT = KPC // 128                # 64 k-tiles per core in phase 2

F32 = mybir.dt.float32
BF16 = mybir.dt.bfloat16

# Run options (test harness may flip these; defaults are what grading uses).
TRACE = False
TRACE_INFO = {}
TRACE_RES = {}

_cache = {}


def _nc():
    return bacc.Bacc(
        "TRN2", target_bir_lowering=False, debug=False, num_devices=NCORES
    )


def _build_phase1():
    """Per-core: x shard -> raw feats [BPC, M*C] (bf16).

    Inputs (host-permuted so every DMA descriptor is a contiguous >=4KB run):
      xp  [128, 4, CT, 2, HW]  batch pairs, c = p*CT + t
      xta [128, BPC, C]        x^T rows hw=0:128
      xtb [ 68, BPC, C]        x^T rows hw=128:196
      wat [128, CT, M]         Wa^T in the same permuted-c layout
      ba  [M, 1]

    All loads ride the sync queue in exact consumption order (a single
    HWDGE queue sustains the full ~360 GB/s); feats stores ride gpsimd and
    are emitted last so no load ever recycles a store's DMA semaphore
    (the pool has only ~8 and a load waiting on a compute-gated store
    stalls the whole ring).
    """
    nc = _nc()
    xp = nc.dram_tensor("xp", [128, 4, CT, 2, HW], BF16, kind="ExternalInput")
    xta = nc.dram_tensor("xta", [128, BPC, C], BF16, kind="ExternalInput")
    xtb = nc.dram_tensor("xtb", [68, BPC, C], BF16, kind="ExternalInput")
    wat = nc.dram_tensor("wat", [128, CT, M], BF16, kind="ExternalInput")
    # ba padded to 512B per partition: a [32,1] fp32 transfer would emit
    # 4-byte descriptors (sub-512B -> SDMA read-modify-write) that stall
    # every engine's FIFO ring for ~10us
    ba = nc.dram_tensor("ba", [M, 128], F32, kind="ExternalInput")
    feats = nc.dram_tensor("feats", [BPC, M * C], BF16, kind="ExternalOutput")

    with TileContext(nc) as tc:
        with (
            tc.tile_pool(name="const", bufs=1) as const,
            tc.tile_pool(name="xpool", bufs=3) as xpool,
            tc.tile_pool(name="xtapool", bufs=3) as xtapool,
            tc.tile_pool(name="xtbpool", bufs=3) as xtbpool,
            tc.tile_pool(name="apool", bufs=3) as apool,
            tc.tile_pool(name="atpool", bufs=4) as atpool,
            tc.tile_pool(name="fpool", bufs=2) as fpool,
            tc.tile_pool(name="ps_att", bufs=2, space="PSUM") as ps_att,
            tc.tile_pool(name="ps_tr", bufs=1, space="PSUM") as ps_tr,
            tc.tile_pool(name="ps_bap", bufs=2, space="PSUM") as ps_bap,
        ):
            # PE warm-up source (memset on gpsimd before identity/stores)
            warm_sb = const.tile([128, 512], BF16)
            nc.gpsimd.memset(warm_sb, 0.0)
            ident = const.tile([M, M], BF16)
            make_identity(nc, ident)

            # loads, sync queue, consumption order
            wat_sb = const.tile([128, CT, M], BF16)
            nc.sync.dma_start(out=wat_sb, in_=wat.ap())
            ba_sb = const.tile([M, 128], F32)
            nc.sync.dma_start(out=ba_sb, in_=ba.ap())
            xps, xtas, xtbs = [], [], []
            for p in range(4):
                x_p = xpool.tile(
                    [128, CT, 2, HW], BF16, tag="xp", name=f"xp{p}"
                )
                nc.sync.dma_start(out=x_p, in_=xp.ap()[:, p])
                xps.append(x_p)
                xta_p = xtapool.tile(
                    [128, 2, C], BF16, tag="xta", name=f"xta{p}"
                )
                nc.sync.dma_start(
                    out=xta_p, in_=xta.ap()[:, 2 * p : 2 * p + 2]
                )
                # xtb covers only 68 partitions (~9/16 SDMA engines), so it
                # rides the scalar queue concurrently with the sync stream
                xtb_p = xtbpool.tile([68, 2, C], BF16, tag="xtb", name=f"xtb{p}")
                nc.scalar.dma_start(
                    out=xtb_p, in_=xtb.ap()[:, 2 * p : 2 * p + 2]
                )
                xtas.append(xta_p)
                xtbs.append(xtb_p)

            # keep the PE busy (and the HAM clock-gate open) during the
            # initial DMA ramp: one long junk accumulation chain.
            w_ps = ps_att.tile([128, 512], F32, tag="att", name="warm")
            for i in range(16):
                nc.tensor.matmul(
                    w_ps, lhsT=warm_sb[:, 0:128], rhs=warm_sb,
                    start=(i == 0), stop=(i == 15),
                )

            a_of = {}

            def emit_e1_pair(i):
                att_ps = ps_att.tile([M, 2, HW], F32, tag="att", name=f"attp{i}")
                for ct in range(CT):
                    nc.tensor.matmul(
                        att_ps,
                        lhsT=wat_sb[:, ct, :],
                        rhs=xps[i][:, ct, :, :],
                        start=(ct == 0),
                        stop=(ct == CT - 1),
                    )
                a_sb = apool.tile(
                    [M, 2, HW], BF16, tag="a_sb", name=f"a_sbp{i}"
                )
                nc.scalar.activation(
                    out=a_sb, in_=att_ps,
                    func=mybir.ActivationFunctionType.Sigmoid,
                    bias=ba_sb[:, 0:1],
                )
                a_of[2 * i] = (a_sb, 0)
                a_of[2 * i + 1] = (a_sb, 1)

            ats = {}

            def emit_tr(b):
                a_sb, b2 = a_of[b]
                ata_ps = ps_tr.tile([128, M], BF16, tag="ata")
                nc.tensor.transpose(ata_ps, a_sb[:, b2, 0:128], ident)
                ata = atpool.tile([128, M], BF16, tag="ata_sb")
                nc.scalar.copy(out=ata, in_=ata_ps)
                atb_ps = ps_tr.tile([68, M], BF16, tag="atb")
                nc.tensor.transpose(atb_ps, a_sb[:, b2, 128:HW], ident)
                atb = atpool.tile([68, M], BF16, tag="atb_sb")
                nc.vector.tensor_copy(out=atb, in_=atb_ps)
                ats[b] = (ata, atb)

            fqs = {}

            def emit_e2(b):
                ata, atb = ats[b]
                xta_b = xtas[b // 2][:, b % 2]
                xtb_b = xtbs[b // 2][:, b % 2]
                if b % 4 == 0:
                    fqs[b // 4] = fpool.tile(
                        [4 * M, C], BF16, tag="featsq", name=f"fq{b // 4}"
                    )
                featsq = fqs[b // 4]
                r0 = M * (b % 4)
                bap_ps = [
                    ps_bap.tile([M, 1024], F32, tag="bap", name=f"bap{b}_{h}")
                    for h in range(2)
                ]
                for h in range(2):
                    for nt in range(2):
                        c0 = 1024 * h + 512 * nt
                        nc.tensor.matmul(
                            bap_ps[h][:, 512 * nt : 512 * (nt + 1)],
                            lhsT=ata, rhs=xta_b[:, c0 : c0 + 512],
                            start=True, stop=False,
                        )
                for h in range(2):
                    for nt in range(2):
                        c0 = 1024 * h + 512 * nt
                        nc.tensor.matmul(
                            bap_ps[h][:, 512 * nt : 512 * (nt + 1)],
                            lhsT=atb, rhs=xtb_b[:, c0 : c0 + 512],
                            start=False, stop=True,
                        )
                # parallel drains: vector gets half 0, scalar gets half 1
                nc.vector.tensor_copy(
                    out=featsq[r0 : r0 + M, 0:1024], in_=bap_ps[0]
                )
                nc.scalar.copy(
                    out=featsq[r0 : r0 + M, 1024:2048], in_=bap_ps[1]
                )

            # PE program order: einsum1 of the next pair is interleaved so
            # the PE never waits on Scalar/Vector round trips.
            emit_e1_pair(0)
            emit_tr(0)
            emit_e2(0)
            emit_tr(1)
            emit_e2(1)
            emit_e1_pair(1)
            emit_tr(2)
            emit_e2(2)
            emit_tr(3)
            emit_e2(3)
            emit_e1_pair(2)
            emit_tr(4)
            emit_e2(4)
            emit_tr(5)
            emit_e2(5)
            emit_e1_pair(3)
            emit_tr(6)
            emit_e2(6)
            emit_tr(7)
            emit_e2(7)

            # quad stores (128 partitions, 4KB descriptors), gpsimd queue,
            # emitted after every load
            for q in range(2):
                nc.gpsimd.dma_start(
                    out=feats.ap()[4 * q : 4 * q + 4].rearrange(
                        "b (m c) -> (b m) c", m=M
                    ),
                    in_=fqs[q],
                )
    nc.compile()
    return nc


def _build_phase2():
    """Per-core: featsT slice (partition-major, bf16) x WcT slice (bf16)
    -> partial [B, NCLS] (fp32)."""
    nc = _nc()
    ft = nc.dram_tensor("ft", [128, KT, B], BF16, kind="ExternalInput")
    wct = nc.dram_tensor("wct", [128, KT, NCLS], BF16, kind="ExternalInput")
    part = nc.dram_tensor("part", [B, NCLS], F32, kind="ExternalOutput")

    # 16-k-tile chunks give 12.7KB per-partition descriptors (near line
    # rate per SDMA engine); the last chunks shrink so the final
    # chunk-completion latency + its matmuls are a short tail.  All loads
    # on sync in consumption order; ft halves interleaved.
    CHUNKS = [16, 16, 16, 8, 4, 2, 2]
    FSTEP = 32

    with TileContext(nc) as tc:
        with (
            tc.tile_pool(name="const", bufs=1) as const,
            tc.tile_pool(name="fpool", bufs=2) as fpool,
            tc.tile_pool(name="wpool", bufs=len(CHUNKS)) as wpool,
            tc.tile_pool(name="opool", bufs=1) as opool,
            tc.tile_pool(name="ps_out", bufs=1, space="PSUM") as ps_out,
            tc.tile_pool(name="ps_warm", bufs=1, space="PSUM") as ps_warm,
        ):
            warm_sb = const.tile([128, 512], BF16)
            nc.gpsimd.memset(warm_sb, 0.0)

            ft_sb = []

            def load_ft(i):
                t = fpool.tile([128, FSTEP, B], BF16, tag="ft", name=f"ft{i}")
                nc.sync.dma_start(
                    out=t, in_=ft.ap()[:, i * FSTEP : (i + 1) * FSTEP]
                )
                ft_sb.append(t)

            load_ft(0)
            w_sbs = []
            k0 = 0
            for kc, ch in enumerate(CHUNKS):
                if k0 + ch > FSTEP * len(ft_sb):
                    load_ft(len(ft_sb))
                w_sb = wpool.tile(
                    [128, ch, NCLS], BF16, tag=f"w{kc}", bufs=1, name=f"w{kc}"
                )
                nc.sync.dma_start(out=w_sb, in_=wct.ap()[:, k0 : k0 + ch])
                w_sbs.append((k0, w_sb))
                k0 += ch

            # junk accumulation chain keeps the PE warm until the first
            # (16-k-tile) chunk lands
            w_ps = ps_warm.tile([128, 512], F32, tag="warm", name="wm")
            for i in range(26):
                nc.tensor.matmul(
                    w_ps, lhsT=warm_sb[:, 0:128], rhs=warm_sb,
                    start=(i == 0), stop=(i == 25),
                )

            out_ps = ps_out.tile([B, NCLS], F32)
            for k0, w_sb in w_sbs:
                for kl in range(w_sb.shape[1]):
                    kt = k0 + kl
                    nc.tensor.matmul(
                        out_ps,
                        lhsT=ft_sb[kt // FSTEP][:, kt % FSTEP, :],
                        rhs=w_sb[:, kl, :],
                        start=(kt == 0),
                        stop=(kt == KT - 1),
                    )
            out_sb = opool.tile([B, NCLS], F32)
            nc.vector.tensor_copy(out=out_sb, in_=out_ps)
            nc.scalar.dma_start(out=part.ap(), in_=out_sb)
    nc.compile()
    return nc


def _install_ntff_hook():
    import types

    import trn_agent_boot.trn_boot as tb
    import concourse.bass_utils as bu

    hook = tb._ntff_profile_via_ctypes("/opt/axon/libaxon_pjrt.so")
    mod = types.ModuleType("antenv.axon_hooks")
    mod.get_axon_ntff_profile_hook = lambda: hook
    sys.modules["antenv.axon_hooks"] = mod
    bu.upload_artifacts = lambda tmpdir: "(skipped)"


def _run(nc, in_maps, label):
    core_ids = list(range(NCORES))
    if TRACE:
        _install_ntff_hook()
        res = run_bass_kernel_spmd(nc, in_maps, core_ids, trace=True)
        TRACE_INFO[label] = res.exec_time_ns
        TRACE_RES[label] = res
    else:
        res = run_bass_kernel_spmd(nc, in_maps, core_ids)
    return res.results


def kernel(x, Wa, ba, Wc, bc):
    import ml_dtypes

    bf16 = np.dtype(ml_dtypes.bfloat16)
    x3 = np.ascontiguousarray(x, dtype=np.float32).reshape(B, C, HW)
    xb = x3.astype(bf16)
    x4 = xb.reshape(B, 128, CT, HW)
    # xp[p, i, t, b2, hw] with c = p*CT + t
    xps = [
        np.ascontiguousarray(
            x4[i * BPC : (i + 1) * BPC]
            .reshape(4, 2, 128, CT, HW)
            .transpose(2, 0, 3, 1, 4)
        )
        for i in range(NCORES)
    ]
    xt = xb.transpose(2, 0, 1)  # [HW, B, C]
    xta = np.ascontiguousarray(xt[0:128])
    xtb = np.ascontiguousarray(xt[128:196])
    # wat[p, t, m] = Wa[m, p*CT + t] — matches the kernel's permuted c layout
    wat = np.ascontiguousarray(Wa.T, dtype=np.float32).astype(bf16).reshape(
        128, CT, M
    )
    ba2 = np.ascontiguousarray(
        np.broadcast_to(
            np.asarray(ba, dtype=np.float32).reshape(M, 1), (M, 128)
        )
    )
    wct = np.ascontiguousarray(Wc.T, dtype=np.float32).astype(bf16)  # [KTOT, NCLS]

    if "p1" not in _cache:
        _cache["p1"] = _build_phase1()
    if "p2" not in _cache:
        _cache["p2"] = _build_phase2()

    in1 = [
        {
            "xp": xps[i],
            "xta": xta[:, i * BPC : (i + 1) * BPC],
            "xtb": xtb[:, i * BPC : (i + 1) * BPC],
            "wat": wat,
            "ba": ba2,
        }
        for i in range(NCORES)
    ]
    res1 = _run(_cache["p1"], in1, "phase1")
    feats = np.concatenate([r["feats"] for r in res1], axis=0)  # [B, KTOT] bf16

    # ft[p, t, b] = feats[b, kslice + t*128 + p] (partition-major, bf16)
    featsT = np.ascontiguousarray(feats.T)  # [KTOT, B]
    in2 = [
        {
            "ft": np.ascontiguousarray(
                featsT[i * KPC : (i + 1) * KPC].reshape(KT, 128, B).transpose(
                    1, 0, 2
                )
            ),
            "wct": np.ascontiguousarray(
                wct[i * KPC : (i + 1) * KPC].reshape(KT, 128, NCLS).transpose(
                    1, 0, 2
                )
            ),
        }
        for i in range(NCORES)
    ]
    res2 = _run(_cache["p2"], in2, "phase2")
    parts = np.stack([r["part"] for r in res2], axis=0)  # [NCORES, B, NCLS]

    logits = parts.sum(axis=0) / float(HW) + np.asarray(bc, dtype=np.float32)
    return logits.astype(np.float32)


# revision 13
# speedup vs baseline: 1.0833x; 1.0711x over previous
"""BAP classifier (attention-pooling + linear head) on 8 TRN2 NeuronCores.

Pipeline (reference math):
    A    = sigmoid(einsum('bchw,mc->bmhw', x, Wa) + ba)     # attention maps
    bap  = einsum('bchw,bmhw->bmc', x, A) / (H*W)           # attn-weighted pool
    out  = bap.reshape(B, M*C) @ Wc.T + bc                  # linear head

Sharding:
  Phase 1 — data-parallel over batch (8 batches/core): each core computes
    raw feats rows [8, M*C] (un-normalized bap, transposed per batch on chip).
  Phase 2 — Wc column-parallel (8192 columns of the M*C dim per core): each
    core computes a partial [B, NCLS] logit; host sums partials, applies the
    1/(H*W) scale and bias.

Compute dtype is bf16 on the TensorEngine with fp32 PSUM accumulation
(rel err vs the fp32 reference lands ~3e-3).
"""
import sys

if "/opt/trn_rl_repo" not in sys.path:
    sys.path.insert(0, "/opt/trn_rl_repo")

import numpy as np

import concourse.bacc as bacc
import concourse.mybir as mybir
from concourse.tile import TileContext
from concourse.bass_utils import run_bass_kernel_spmd
from concourse.masks import make_identity

B, C, H, W = 64, 2048, 14, 14
HW = H * W                     # 196
M, NCLS = 32, 396
NCORES = 8
BPC = B // NCORES              # 8 batches per core
CT = C // 128                  # 16 c-chunks
KTOT = M * C                   # 65536
KPC = KTOT // NCORES           # 8192 Wc columns per core
KT = KPC // 128                # 64 k-tiles per core in phase 2

F32 = mybir.dt.float32
F32R = mybir.dt.float32r
BF16 = mybir.dt.bfloat16

# Run options (test harness may flip these; defaults are what grading uses).
TRACE = False
TRACE_INFO = {}
TRACE_RES = {}

_cache = {}


def _nc():
    return bacc.Bacc(
        "TRN2", target_bir_lowering=False, debug=False, num_devices=NCORES
    )


def _build_phase1():
    """Per-core: x_shard (bf16) [BPC, C, HW] -> raw feats [BPC, M*C].

    c is loaded with the permuted mapping c = p*CT + t (p = partition,
    t = chunk) so every natural-load descriptor is one contiguous 6.1KB run;
    wat arrives host-permuted to the same mapping.  x^T for the BAP einsum
    is supplied pre-transposed by the host ([BPC, HW, C]) and read as two
    plain partition-tiles per batch (hw 0:128 and 128:196) — no on-device
    transposes, so all DMA queues run in parallel.
    """
    nc = _nc()
    x = nc.dram_tensor("x", [BPC, C, HW], BF16, kind="ExternalInput")
    xt = nc.dram_tensor("xt", [BPC, HW, C], BF16, kind="ExternalInput")
    wat = nc.dram_tensor("wat", [128, CT, M], BF16, kind="ExternalInput")
    ba = nc.dram_tensor("ba", [M, 1], F32, kind="ExternalInput")
    feats = nc.dram_tensor("feats", [BPC, M * C], BF16, kind="ExternalOutput")

    with TileContext(nc) as tc:
        with (
            tc.tile_pool(name="const", bufs=1) as const,
            tc.tile_pool(name="xpool", bufs=4) as xpool,
            tc.tile_pool(name="xtpool", bufs=8) as xtpool,
            tc.tile_pool(name="apool", bufs=4) as apool,
            tc.tile_pool(name="atpool", bufs=4) as atpool,
            tc.tile_pool(name="fpool", bufs=4) as fpool,
            tc.tile_pool(name="ps_att", bufs=2, space="PSUM") as ps_att,
            tc.tile_pool(name="ps_tr", bufs=1, space="PSUM") as ps_tr,
            tc.tile_pool(name="ps_bap", bufs=4, space="PSUM") as ps_bap,
        ):
            wat_sb = const.tile([128, CT, M], BF16)
            nc.sync.dma_start(out=wat_sb, in_=wat.ap())
            ba_sb = const.tile([M, 1], F32)
            nc.sync.dma_start(out=ba_sb, in_=ba.ap())
            ident = const.tile([M, M], BF16)
            make_identity(nc, ident)

            # hoist every input load to the top of the Sync queue: all four
            # x pairs first (einsum1 streams + PE warms early), then the x^T
            # tiles in batch order (einsum2 tail is just the last batch).
            x_pairs = []
            xts = []
            for pr in range(BPC // 2):
                x_pair = xpool.tile(
                    [128, 2, CT, HW], BF16, tag="x_pair", name=f"x_pair{pr}"
                )
                nc.sync.dma_start(
                    out=x_pair,
                    in_=x.ap()[2 * pr : 2 * pr + 2].rearrange(
                        "b (p t) f -> p b t f", t=CT
                    ),
                )
                x_pairs.append(x_pair)
            for b in range(BPC):
                xta = xtpool.tile([128, C], BF16, tag="xta", name=f"xta{b}")
                xtb = xtpool.tile([68, C], BF16, tag="xtb", name=f"xtb{b}")
                nc.sync.dma_start(out=xta, in_=xt.ap()[b, 0:128, :])
                nc.sync.dma_start(out=xtb, in_=xt.ap()[b, 128:196, :])
                xts.append((xta, xtb))

            # einsum1 for all pairs first: one long PE burst (warms the
            # HAM clock gate) while the x^T tiles are still streaming in.
            a_sbs = []

            def emit_bap(pr):
                    a_sb = a_sbs[pr]
                    # feats staging for the pair: partition = 32*b2 + m
                    featsq = fpool.tile([64, C], BF16, tag="featsq", name=f"featsq{pr}")
                    for b2 in range(2):
                        b = 2 * pr + b2
                        xta, xtb = xts[b]

                        # A^T chunks via PE transpose
                        ata_ps = ps_tr.tile([128, M], BF16, tag="ata")
                        nc.tensor.transpose(
                            ata_ps, a_sb[:, b2, 0:128], ident[0:M, 0:M]
                        )
                        ata = atpool.tile([128, M], BF16, tag="ata_sb")
                        nc.scalar.copy(out=ata, in_=ata_ps)

                        atb_ps = ps_tr.tile([68, M], BF16, tag="atb")
                        nc.tensor.transpose(
                            atb_ps, a_sb[:, b2, 128:196], ident[0:M, 0:M]
                        )
                        atb = atpool.tile([68, M], BF16, tag="atb_sb")
                        nc.scalar.copy(out=atb, in_=atb_ps)

                        # einsum2: bapT[m, c] = sum_hw A[m,hw] * x[c,hw]
                        # All four 512-wide output chunks per A^T half so the
                        # PE loads each stationary operand once.
                        bap_ps = [
                            ps_bap.tile([M, 512], F32, tag="bap", name=f"bap_ps{nt}")
                            for nt in range(4)
                        ]
                        for nt in range(4):
                            nc.tensor.matmul(
                                bap_ps[nt],
                                lhsT=ata,
                                rhs=xta[:, 512 * nt : 512 * (nt + 1)],
                                start=True,
                                stop=False,
                            )
                        for nt in range(4):
                            nc.tensor.matmul(
                                bap_ps[nt],
                                lhsT=atb,
                                rhs=xtb[:, 512 * nt : 512 * (nt + 1)],
                                start=False,
                                stop=True,
                            )
                        row = 32 * b2
                        for nt in range(4):
                            nc.vector.tensor_copy(
                                out=featsq[
                                    row : row + 32,
                                    512 * nt : 512 * (nt + 1),
                                ],
                                in_=bap_ps[nt],
                            )
                    nc.scalar.dma_start(
                        out=feats.ap()[2 * pr : 2 * pr + 2].rearrange(
                            "b (m c) -> (b m) c", m=M
                        ),
                        in_=featsq,
                    )

            for pr in range(BPC // 2):
                x_pair = x_pairs[pr]
                att_ps = ps_att.tile([M, 2, HW], F32, tag="att", name=f"att{pr}")
                for ct in range(CT):
                    nc.tensor.matmul(
                        att_ps,
                        lhsT=wat_sb[:, ct, :],
                        rhs=x_pair[:, :, ct, :],
                        start=(ct == 0),
                        stop=(ct == CT - 1),
                    )
                a_sb = apool.tile([M, 2, HW], BF16, tag="a_sb", name=f"a_sb{pr}")
                nc.scalar.activation(
                    out=a_sb,
                    in_=att_ps,
                    func=mybir.ActivationFunctionType.Sigmoid,
                    bias=ba_sb,
                )
                a_sbs.append(a_sb)

                if pr >= 1:
                    emit_bap(pr - 1)

            emit_bap(BPC // 2 - 1)
    nc.compile()
    return nc


def _build_phase2():
    """Per-core: featsT slice (partition-major, bf16) x WcT slice (bf16)
    -> partial [B, NCLS] (fp32)."""
    nc = _nc()
    ft = nc.dram_tensor("ft", [128, KT, B], BF16, kind="ExternalInput")
    wct = nc.dram_tensor("wct", [128, KT, NCLS], BF16, kind="ExternalInput")
    part = nc.dram_tensor("part", [B, NCLS], F32, kind="ExternalOutput")

    # 16-k-tile chunks give 12.7KB per-partition descriptors (near line
    # rate per SDMA engine); the last chunks shrink so the final
    # chunk-completion latency + its matmuls are a short tail.  All loads
    # on sync in consumption order; ft halves interleaved.
    CHUNKS = [16, 16, 16, 8, 4, 2, 2]
    FSTEP = 32

    with TileContext(nc) as tc:
        with (
            tc.tile_pool(name="const", bufs=1) as const,
            tc.tile_pool(name="fpool", bufs=2) as fpool,
            tc.tile_pool(name="wpool", bufs=len(CHUNKS)) as wpool,
            tc.tile_pool(name="opool", bufs=1) as opool,
            tc.tile_pool(name="ps_out", bufs=1, space="PSUM") as ps_out,
            tc.tile_pool(name="ps_warm", bufs=1, space="PSUM") as ps_warm,
        ):
            warm_sb = const.tile([128, 512], BF16)
            nc.gpsimd.memset(warm_sb, 0.0)

            ft_sb = []

            def load_ft(i):
                t = fpool.tile([128, FSTEP, B], BF16, tag="ft", name=f"ft{i}")
                nc.sync.dma_start(
                    out=t, in_=ft.ap()[:, i * FSTEP : (i + 1) * FSTEP]
                )
                ft_sb.append(t)

            load_ft(0)
            w_sbs = []
            k0 = 0
            for kc, ch in enumerate(CHUNKS):
                if k0 + ch > FSTEP * len(ft_sb):
                    load_ft(len(ft_sb))
                w_sb = wpool.tile(
                    [128, ch, NCLS], BF16, tag=f"w{kc}", bufs=1, name=f"w{kc}"
                )
                nc.sync.dma_start(out=w_sb, in_=wct.ap()[:, k0 : k0 + ch])
                w_sbs.append((k0, w_sb))
                k0 += ch

            # junk accumulation chain keeps the PE warm until the first
            # (16-k-tile) chunk lands
            w_ps = ps_warm.tile([128, 512], F32, tag="warm", name="wm")
            for i in range(26):
                nc.tensor.matmul(
                    w_ps, lhsT=warm_sb[:, 0:128], rhs=warm_sb,
                    start=(i == 0), stop=(i == 25),
                )

            out_ps = ps_out.tile([B, NCLS], F32)
            for k0, w_sb in w_sbs:
                for kl in range(w_sb.shape[1]):
                    kt = k0 + kl
                    nc.tensor.matmul(
                        out_ps,
                        lhsT=ft_sb[kt // FSTEP][:, kt % FSTEP, :],
                        rhs=w_sb[:, kl, :],
                        start=(kt == 0),
                        stop=(kt == KT - 1),
                    )
            out_sb = opool.tile([B, NCLS], F32)
            nc.vector.tensor_copy(out=out_sb, in_=out_ps)
            nc.scalar.dma_start(out=part.ap(), in_=out_sb)
    nc.compile()
    return nc


def _install_ntff_hook():
    import types

    import trn_agent_boot.trn_boot as tb
    import concourse.bass_utils as bu

    hook = tb._ntff_profile_via_ctypes("/opt/axon/libaxon_pjrt.so")
    mod = types.ModuleType("antenv.axon_hooks")
    mod.get_axon_ntff_profile_hook = lambda: hook
    sys.modules["antenv.axon_hooks"] = mod
    bu.upload_artifacts = lambda tmpdir: "(skipped)"


def _run(nc, in_maps, label):
    core_ids = list(range(NCORES))
    if TRACE:
        _install_ntff_hook()
        res = run_bass_kernel_spmd(nc, in_maps, core_ids, trace=True)
        TRACE_INFO[label] = res.exec_time_ns
        TRACE_RES[label] = res
    else:
        res = run_bass_kernel_spmd(nc, in_maps, core_ids)
    return res.results


def kernel(x, Wa, ba, Wc, bc):
    import ml_dtypes

    bf16 = np.dtype(ml_dtypes.bfloat16)
    x3 = np.ascontiguousarray(x, dtype=np.float32).reshape(B, C, HW)
    x = x3.astype(bf16)
    xt = np.ascontiguousarray(x.transpose(0, 2, 1))  # [B, HW, C] bf16
    # wat[p, t, m] = Wa[m, p*CT + t] — matches the kernel's permuted c layout
    wat = np.ascontiguousarray(Wa.T, dtype=np.float32).astype(bf16).reshape(
        128, CT, M
    )
    ba2 = np.ascontiguousarray(ba, dtype=np.float32).reshape(M, 1)
    wct = np.ascontiguousarray(Wc.T, dtype=np.float32).astype(bf16)  # [KTOT, NCLS]

    if "p1" not in _cache:
        _cache["p1"] = _build_phase1()
    if "p2" not in _cache:
        _cache["p2"] = _build_phase2()

    in1 = [
        {
            "x": x[i * BPC : (i + 1) * BPC],
            "xt": xt[i * BPC : (i + 1) * BPC],
            "wat": wat,
            "ba": ba2,
        }
        for i in range(NCORES)
    ]
    res1 = _run(_cache["p1"], in1, "phase1")
    feats = np.concatenate([r["feats"] for r in res1], axis=0)  # [B, KTOT] bf16

    # ft[p, t, b] = feats[b, kslice + t*128 + p] (partition-major, bf16)
    featsT = np.ascontiguousarray(feats.T)  # [KTOT, B]
    in2 = [
        {
            "ft": np.ascontiguousarray(
                featsT[i * KPC : (i + 1) * KPC].reshape(KT, 128, B).transpose(
                    1, 0, 2
                )
            ),
            "wct": np.ascontiguousarray(
                wct[i * KPC : (i + 1) * KPC].reshape(KT, 128, NCLS).transpose(
                    1, 0, 2
                )
            ),
        }
        for i in range(NCORES)
    ]
    res2 = _run(_cache["p2"], in2, "phase2")
    parts = np.stack([r["part"] for r in res2], axis=0)  # [NCORES, B, NCLS]

    logits = parts.sum(axis=0) / float(HW) + np.asarray(bc, dtype=np.float32)
    return logits.astype(np.float32)



# revision 14
# speedup vs baseline: 1.1164x; 1.0305x over previous
"""BAP classifier (attention-pooling + linear head) on 8 TRN2 NeuronCores.

Pipeline (reference math):
    A    = sigmoid(einsum('bchw,mc->bmhw', x, Wa) + ba)     # attention maps
    bap  = einsum('bchw,bmhw->bmc', x, A) / (H*W)           # attn-weighted pool
    out  = bap.reshape(B, M*C) @ Wc.T + bc                  # linear head

Sharding:
  Phase 1 — data-parallel over batch (8 batches/core): each core computes
    raw feats rows [8, M*C] (un-normalized bap, transposed per batch on chip).
  Phase 2 — Wc column-parallel (8192 columns of the M*C dim per core): each
    core computes a partial [B, NCLS] logit; host sums partials, applies the
    1/(H*W) scale and bias.

Compute dtype is bf16 on the TensorEngine with fp32 PSUM accumulation
(rel err vs the fp32 reference lands ~3e-3).
"""
import sys

if "/opt/trn_rl_repo" not in sys.path:
    sys.path.insert(0, "/opt/trn_rl_repo")

import numpy as np

import concourse.bacc as bacc
import concourse.mybir as mybir
from concourse.tile import TileContext
from concourse.bass_utils import run_bass_kernel_spmd
from concourse.masks import make_identity

B, C, H, W = 64, 2048, 14, 14
HW = H * W                     # 196
M, NCLS = 32, 396
NCORES = 8
BPC = B // NCORES              # 8 batches per core
CT = C // 128                  # 16 c-chunks
KTOT = M * C                   # 65536
KPC = KTOT // NCORES           # 8192 Wc columns per core
KT = KPC // 128                # 64 k-tiles per core in phase 2

F32 = mybir.dt.float32
F32R = mybir.dt.float32r
BF16 = mybir.dt.bfloat16

# Run options (test harness may flip these; defaults are what grading uses).
TRACE = False
TRACE_INFO = {}
TRACE_RES = {}

_cache = {}


def _nc():
    return bacc.Bacc(
        "TRN2", target_bir_lowering=False, debug=False, num_devices=NCORES
    )


def _build_phase1():
    """Per-core: x_shard (bf16) [BPC, C, HW] -> raw feats [BPC, M*C].

    c is loaded with the permuted mapping c = p*CT + t (p = partition,
    t = chunk) so every natural-load descriptor is one contiguous 6.1KB run;
    wat arrives host-permuted to the same mapping.  x^T for the BAP einsum
    is supplied pre-transposed by the host ([BPC, HW, C]) and read as two
    plain partition-tiles per batch (hw 0:128 and 128:196) — no on-device
    transposes, so all DMA queues run in parallel.
    """
    nc = _nc()
    x = nc.dram_tensor("x", [BPC, C, HW], BF16, kind="ExternalInput")
    xt = nc.dram_tensor("xt", [BPC, HW, C], BF16, kind="ExternalInput")
    wat = nc.dram_tensor("wat", [128, CT, M], BF16, kind="ExternalInput")
    # ba padded to 512B per partition (4-byte descriptors force SDMA
    # read-modify-write and stall the ring)
    ba = nc.dram_tensor("ba", [M, 128], F32, kind="ExternalInput")
    feats = nc.dram_tensor("feats", [BPC, M * C], BF16, kind="ExternalOutput")

    with TileContext(nc) as tc:
        with (
            tc.tile_pool(name="const", bufs=1) as const,
            tc.tile_pool(name="xpool", bufs=4) as xpool,
            tc.tile_pool(name="xtpool", bufs=8) as xtpool,
            tc.tile_pool(name="apool", bufs=4) as apool,
            tc.tile_pool(name="atpool", bufs=4) as atpool,
            tc.tile_pool(name="fpool", bufs=4) as fpool,
            tc.tile_pool(name="ps_att", bufs=2, space="PSUM") as ps_att,
            tc.tile_pool(name="ps_tr", bufs=1, space="PSUM") as ps_tr,
            tc.tile_pool(name="ps_bap", bufs=4, space="PSUM") as ps_bap,
        ):
            wat_sb = const.tile([128, CT, M], BF16)
            nc.sync.dma_start(out=wat_sb, in_=wat.ap())
            ba_sb = const.tile([M, 128], F32)
            nc.sync.dma_start(out=ba_sb, in_=ba.ap())
            warm_sb = const.tile([128, 512], BF16)
            nc.gpsimd.memset(warm_sb, 0.0)
            ident = const.tile([M, M], BF16)
            make_identity(nc, ident)
            # junk accumulation chain keeps the PE (HAM clock-gate) warm
            # through the initial DMA ramp so einsum1 starts at 2.4 GHz
            w_ps = ps_att.tile([128, 512], F32, tag="att", name="warm")
            for i in range(20):
                nc.tensor.matmul(
                    w_ps, lhsT=warm_sb[:, 0:128], rhs=warm_sb,
                    start=(i == 0), stop=(i == 19),
                )

            # hoist every input load to the top of the Sync queue: all four
            # x pairs first (einsum1 streams + PE warms early), then the x^T
            # tiles in batch order (einsum2 tail is just the last batch).
            x_pairs = []
            xts = []
            for pr in range(BPC // 2):
                x_pair = xpool.tile(
                    [128, 2, CT, HW], BF16, tag="x_pair", name=f"x_pair{pr}"
                )
                nc.sync.dma_start(
                    out=x_pair,
                    in_=x.ap()[2 * pr : 2 * pr + 2].rearrange(
                        "b (p t) f -> p b t f", t=CT
                    ),
                )
                x_pairs.append(x_pair)
            for b in range(BPC):
                xta = xtpool.tile([128, C], BF16, tag="xta", name=f"xta{b}")
                xtb = xtpool.tile([68, C], BF16, tag="xtb", name=f"xtb{b}")
                nc.sync.dma_start(out=xta, in_=xt.ap()[b, 0:128, :])
                nc.sync.dma_start(out=xtb, in_=xt.ap()[b, 128:196, :])
                xts.append((xta, xtb))

            # einsum1 for all pairs first: one long PE burst (warms the
            # HAM clock gate) while the x^T tiles are still streaming in.
            a_sbs = []

            def emit_bap(pr):
                    a_sb = a_sbs[pr]
                    # feats staging for the pair: partition = 32*b2 + m
                    featsq = fpool.tile([64, C], BF16, tag="featsq", name=f"featsq{pr}")
                    for b2 in range(2):
                        b = 2 * pr + b2
                        xta, xtb = xts[b]

                        # A^T chunks via PE transpose
                        ata_ps = ps_tr.tile([128, M], BF16, tag="ata")
                        nc.tensor.transpose(
                            ata_ps, a_sb[:, b2, 0:128], ident[0:M, 0:M]
                        )
                        ata = atpool.tile([128, M], BF16, tag="ata_sb")
                        nc.scalar.copy(out=ata, in_=ata_ps)

                        atb_ps = ps_tr.tile([68, M], BF16, tag="atb")
                        nc.tensor.transpose(
                            atb_ps, a_sb[:, b2, 128:196], ident[0:M, 0:M]
                        )
                        atb = atpool.tile([68, M], BF16, tag="atb_sb")
                        nc.scalar.copy(out=atb, in_=atb_ps)

                        # einsum2: bapT[m, c] = sum_hw A[m,hw] * x[c,hw]
                        # All four 512-wide output chunks per A^T half so the
                        # PE loads each stationary operand once.
                        bap_ps = [
                            ps_bap.tile([M, 512], F32, tag="bap", name=f"bap_ps{nt}")
                            for nt in range(4)
                        ]
                        for nt in range(4):
                            nc.tensor.matmul(
                                bap_ps[nt],
                                lhsT=ata,
                                rhs=xta[:, 512 * nt : 512 * (nt + 1)],
                                start=True,
                                stop=False,
                            )
                        for nt in range(4):
                            nc.tensor.matmul(
                                bap_ps[nt],
                                lhsT=atb,
                                rhs=xtb[:, 512 * nt : 512 * (nt + 1)],
                                start=False,
                                stop=True,
                            )
                        row = 32 * b2
                        for nt in range(4):
                            eng = (
                                nc.vector.tensor_copy
                                if nt < 2
                                else nc.scalar.copy
                            )
                            eng(
                                out=featsq[
                                    row : row + 32,
                                    512 * nt : 512 * (nt + 1),
                                ],
                                in_=bap_ps[nt],
                            )
                    nc.scalar.dma_start(
                        out=feats.ap()[2 * pr : 2 * pr + 2].rearrange(
                            "b (m c) -> (b m) c", m=M
                        ),
                        in_=featsq,
                    )

            for pr in range(BPC // 2):
                x_pair = x_pairs[pr]
                att_ps = ps_att.tile([M, 2, HW], F32, tag="att", name=f"att{pr}")
                for ct in range(CT):
                    nc.tensor.matmul(
                        att_ps,
                        lhsT=wat_sb[:, ct, :],
                        rhs=x_pair[:, :, ct, :],
                        start=(ct == 0),
                        stop=(ct == CT - 1),
                    )
                a_sb = apool.tile([M, 2, HW], BF16, tag="a_sb", name=f"a_sb{pr}")
                nc.scalar.activation(
                    out=a_sb,
                    in_=att_ps,
                    func=mybir.ActivationFunctionType.Sigmoid,
                    bias=ba_sb[:, 0:1],
                )
                a_sbs.append(a_sb)

                if pr >= 1:
                    emit_bap(pr - 1)

            emit_bap(BPC // 2 - 1)
    nc.compile()
    return nc


def _build_phase2():
    """Per-core: featsT slice (partition-major, bf16) x WcT slice (bf16)
    -> partial [B, NCLS] (fp32)."""
    nc = _nc()
    ft = nc.dram_tensor("ft", [128, KT, B], BF16, kind="ExternalInput")
    wct = nc.dram_tensor("wct", [128, KT, NCLS], BF16, kind="ExternalInput")
    part = nc.dram_tensor("part", [B, NCLS], F32, kind="ExternalOutput")

    # 16-k-tile chunks give 12.7KB per-partition descriptors (near line
    # rate per SDMA engine); the last chunks shrink so the final
    # chunk-completion latency + its matmuls are a short tail.  All loads
    # on sync in consumption order; ft halves interleaved.
    CHUNKS = [16, 16, 16, 8, 4, 2, 2]
    FSTEP = 32

    with TileContext(nc) as tc:
        with (
            tc.tile_pool(name="const", bufs=1) as const,
            tc.tile_pool(name="fpool", bufs=2) as fpool,
            tc.tile_pool(name="wpool", bufs=len(CHUNKS)) as wpool,
            tc.tile_pool(name="opool", bufs=1) as opool,
            tc.tile_pool(name="ps_out", bufs=1, space="PSUM") as ps_out,
            tc.tile_pool(name="ps_warm", bufs=1, space="PSUM") as ps_warm,
        ):
            warm_sb = const.tile([128, 512], BF16)
            nc.gpsimd.memset(warm_sb, 0.0)

            ft_sb = []

            def load_ft(i):
                t = fpool.tile([128, FSTEP, B], BF16, tag="ft", name=f"ft{i}")
                nc.sync.dma_start(
                    out=t, in_=ft.ap()[:, i * FSTEP : (i + 1) * FSTEP]
                )
                ft_sb.append(t)

            load_ft(0)
            w_sbs = []
            k0 = 0
            for kc, ch in enumerate(CHUNKS):
                if k0 + ch > FSTEP * len(ft_sb):
                    load_ft(len(ft_sb))
                w_sb = wpool.tile(
                    [128, ch, NCLS], BF16, tag=f"w{kc}", bufs=1, name=f"w{kc}"
                )
                nc.sync.dma_start(out=w_sb, in_=wct.ap()[:, k0 : k0 + ch])
                w_sbs.append((k0, w_sb))
                k0 += ch

            # junk accumulation chain keeps the PE warm until the first
            # (16-k-tile) chunk lands
            w_ps = ps_warm.tile([128, 512], F32, tag="warm", name="wm")
            for i in range(26):
                nc.tensor.matmul(
                    w_ps, lhsT=warm_sb[:, 0:128], rhs=warm_sb,
                    start=(i == 0), stop=(i == 25),
                )

            out_ps = ps_out.tile([B, NCLS], F32)
            for k0, w_sb in w_sbs:
                for kl in range(w_sb.shape[1]):
                    kt = k0 + kl
                    nc.tensor.matmul(
                        out_ps,
                        lhsT=ft_sb[kt // FSTEP][:, kt % FSTEP, :],
                        rhs=w_sb[:, kl, :],
                        start=(kt == 0),
                        stop=(kt == KT - 1),
                    )
            out_sb = opool.tile([B, NCLS], F32)
            nc.vector.tensor_copy(out=out_sb, in_=out_ps)
            nc.scalar.dma_start(out=part.ap(), in_=out_sb)
    nc.compile()
    return nc


def _install_ntff_hook():
    import types

    import trn_agent_boot.trn_boot as tb
    import concourse.bass_utils as bu

    hook = tb._ntff_profile_via_ctypes("/opt/axon/libaxon_pjrt.so")
    mod = types.ModuleType("antenv.axon_hooks")
    mod.get_axon_ntff_profile_hook = lambda: hook
    sys.modules["antenv.axon_hooks"] = mod
    bu.upload_artifacts = lambda tmpdir: "(skipped)"


def _run(nc, in_maps, label):
    core_ids = list(range(NCORES))
    if TRACE:
        _install_ntff_hook()
        res = run_bass_kernel_spmd(nc, in_maps, core_ids, trace=True)
        TRACE_INFO[label] = res.exec_time_ns
        TRACE_RES[label] = res
    else:
        res = run_bass_kernel_spmd(nc, in_maps, core_ids)
    return res.results


def kernel(x, Wa, ba, Wc, bc):
    import ml_dtypes

    bf16 = np.dtype(ml_dtypes.bfloat16)
    x3 = np.ascontiguousarray(x, dtype=np.float32).reshape(B, C, HW)
    x = x3.astype(bf16)
    xt = np.ascontiguousarray(x.transpose(0, 2, 1))  # [B, HW, C] bf16
    # wat[p, t, m] = Wa[m, p*CT + t] — matches the kernel's permuted c layout
    wat = np.ascontiguousarray(Wa.T, dtype=np.float32).astype(bf16).reshape(
        128, CT, M
    )
    ba2 = np.ascontiguousarray(
        np.broadcast_to(
            np.asarray(ba, dtype=np.float32).reshape(M, 1), (M, 128)
        )
    )
    wct = np.ascontiguousarray(Wc.T, dtype=np.float32).astype(bf16)  # [KTOT, NCLS]

    if "p1" not in _cache:
        _cache["p1"] = _build_phase1()
    if "p2" not in _cache:
        _cache["p2"] = _build_phase2()

    in1 = [
        {
            "x": x[i * BPC : (i + 1) * BPC],
            "xt": xt[i * BPC : (i + 1) * BPC],
            "wat": wat,
            "ba": ba2,
        }
        for i in range(NCORES)
    ]
    res1 = _run(_cache["p1"], in1, "phase1")
    feats = np.concatenate([r["feats"] for r in res1], axis=0)  # [B, KTOT] bf16

    # ft[p, t, b] = feats[b, kslice + t*128 + p] (partition-major, bf16)
    featsT = np.ascontiguousarray(feats.T)  # [KTOT, B]
    in2 = [
        {
            "ft": np.ascontiguousarray(
                featsT[i * KPC : (i + 1) * KPC].reshape(KT, 128, B).transpose(
                    1, 0, 2
                )
            ),
            "wct": np.ascontiguousarray(
                wct[i * KPC : (i + 1) * KPC].reshape(KT, 128, NCLS).transpose(
                    1, 0, 2
                )
            ),
        }
        for i in range(NCORES)
    ]
    res2 = _run(_cache["p2"], in2, "phase2")
    parts = np.stack([r["part"] for r in res2], axis=0)  # [NCORES, B, NCLS]

    logits = parts.sum(axis=0) / float(HW) + np.asarray(bc, dtype=np.float32)
    return logits.astype(np.float32)

